# revision 18
# baseline (speedup 1.0000x reference)
"""Fused multi-head attention block (QKV proj + per-head RMSNorm + RoPE +
softmax attention + output proj) on 8 Trainium2 NeuronCores.

Sharding: core c handles (batch b = c//2, head-group hg = c%2 of 8 heads).
Each core computes a partial output projection over its 8 heads; the host
sums the two partials per batch.

Self-contained: hardcodes B=4, T=2048, C=2048, H=16, D=128.
"""

import math
import sys
import types

import numpy as np
import ml_dtypes

import concourse.bass as bass
import concourse.bacc as bacc
import concourse.tile as tile
from concourse import mybir
from concourse.bass_utils import run_bass_kernel_spmd
from concourse.masks import make_identity

BF16 = mybir.dt.bfloat16
F32 = mybir.dt.float32
NP_BF16 = ml_dtypes.bfloat16
AF = mybir.ActivationFunctionType
ALU = mybir.AluOpType
AX = mybir.AxisListType

B, T, C, H, D = 4, 2048, 2048, 16, 128
HL = H // 2  # heads per core
EPS = 1e-6
NCORES = 8


def _bcast_mid(ap2d, n):
    """[P, F] AP -> [P, n, F] AP broadcast along a new middle dim (step 0)."""
    return bass.AP(
        tensor=ap2d.tensor,
        offset=ap2d.offset,
        ap=[ap2d.ap[0], [0, n], ap2d.ap[1]],
    )


def build(T_=T):
    """Build + compile the per-core Bass program (identical on all cores)."""
    nt = T_ // 128  # number of 128-row t-tiles
    ng = T_ // 512  # number of 512-row t-groups (tq chunks)
    ncl = C // 128  # contraction tiles over C

    nc = bacc.Bacc("TRN2", target_bir_lowering=False, debug=False, num_devices=NCORES)

    xt = nc.dram_tensor("xt", [C, T_], BF16, kind="ExternalInput")  # x[b].T
    wq = nc.dram_tensor("wq", [C, 3 * HL * D], BF16, kind="ExternalInput")  # (c, f')
    wp = nc.dram_tensor("wp", [HL, D, C], BF16, kind="ExternalInput")  # (h, dv, o)
    cq = nc.dram_tensor("cq", [T_, D], F32, kind="ExternalInput")
    sq = nc.dram_tensor("sq", [T_, D], F32, kind="ExternalInput")
    ck = nc.dram_tensor("ck", [T_, D], F32, kind="ExternalInput")
    sk = nc.dram_tensor("sk", [T_, D], F32, kind="ExternalInput")
    out = nc.dram_tensor("out", [T_, C], F32, kind="ExternalOutput")

    with tile.TileContext(nc) as tc:
        with (
            tc.tile_pool(name="persist", bufs=1) as persist,
            tc.tile_pool(name="dram", bufs=1, space="DRAM") as dpool,
        ):
            ident = persist.tile([128, 128], BF16)
            make_identity(nc, ident[:])
            ones_b = persist.tile([128, 1], BF16)
            nc.vector.memset(ones_b[:], 1.0)
            ones_f = persist.tile([1, 128], F32)
            nc.vector.memset(ones_f[:], 1.0)

            KT = persist.tile([128, HL, T_], BF16)  # k^T: (d, h, t)
            Vs = persist.tile([128, HL, nt, D], BF16)  # v: (t-part, h, t-tile, dv)
            qt_dram = dpool.tile([HL, D, T_], BF16)  # q^T spill: (h, d, t)

            # ---------- phase 1: QKV proj + RMS norm + RoPE + transposes ----------
            with (
                tc.tile_pool(name="wq_pool", bufs=3) as wq_pool,
                tc.tile_pool(name="x_pool", bufs=2) as x_pool,
                tc.tile_pool(name="cs_pool", bufs=2) as cs_pool,
                tc.tile_pool(name="work", bufs=2) as work,
                tc.tile_pool(name="qk_ps", bufs=2, space="PSUM") as qk_ps,
                tc.tile_pool(name="tp_ps", bufs=2, space="PSUM") as tp_ps,
            ):

                def phase1_post(fg, tt, ps, cos_t, sin_t, ts):
                    """Consume a finished QKV psum tile: norm+rope+transpose
                    (q/k) or copy out (v)."""
                    ps3 = ps[:].rearrange("p (h d) -> p h d", h=HL)
                    if fg == 2:
                        nc.scalar.copy(Vs[:, :, tt, :], ps3)
                        return
                    # RMS norm stats (Square on ACT: DVE can't read two PSUM
                    # operands)
                    sqs = work.tile([128, HL * D], F32, tag="sqs")
                    nc.scalar.activation(sqs[:], ps[:], AF.Square)
                    sums = work.tile([128, HL], F32, tag="sums")
                    nc.vector.tensor_reduce(
                        out=sums[:],
                        in_=sqs[:].rearrange("p (h d) -> p h d", h=HL),
                        axis=AX.X,
                        op=ALU.add,
                    )
                    ms = work.tile([128, HL], F32, tag="ms")
                    nc.vector.tensor_scalar(
                        out=ms[:],
                        in0=sums[:],
                        scalar1=1.0 / D,
                        scalar2=EPS,
                        op0=ALU.mult,
                        op1=ALU.add,
                    )
                    srt = work.tile([128, HL], F32, tag="srt")
                    nc.scalar.sqrt(srt[:], ms[:])
                    rstd = work.tile([128, HL], F32, tag="rstd")
                    nc.vector.reciprocal(rstd[:], srt[:])
                    # RoPE (scale/sign/g folded into cos/sin host-side)
                    cs_ = cos_t[:, ts, :]
                    sn_ = sin_t[:, ts, :]
                    t3a = work.tile([128, HL, D], F32, tag="t3a")
                    nc.vector.tensor_mul(t3a[:], ps3, _bcast_mid(cs_, HL))
                    t3b = work.tile([128, HL, D], F32, tag="t3b")
                    nc.vector.tensor_mul(
                        t3b[:, :, 0:64],
                        ps3[:, :, 64:128],
                        _bcast_mid(sn_[:, 0:64], HL),
                    )
                    nc.vector.tensor_mul(
                        t3b[:, :, 64:128],
                        ps3[:, :, 0:64],
                        _bcast_mid(sn_[:, 64:128], HL),
                    )
                    t3 = work.tile([128, HL, D], F32, tag="t3")
                    nc.vector.tensor_add(t3[:], t3a[:], t3b[:])
                    rbf = work.tile([128, HL, D], BF16, tag="rbf")
                    for h in range(HL):
                        nc.vector.tensor_scalar_mul(
                            rbf[:, h, :], t3[:, h, :], rstd[:, h : h + 1]
                        )
                    # transpose to (d, t) in 2 groups of 4 heads
                    for hp in range(2):
                        tp = tp_ps.tile([128, 4, 128], BF16, tag="tp")
                        for j in range(4):
                            nc.tensor.transpose(
                                tp[:, j, :], rbf[:, hp * 4 + j, :], ident[:]
                            )
                        if fg == 1:
                            nc.scalar.copy(
                                KT[:, hp * 4 : (hp + 1) * 4, tt * 128 : (tt + 1) * 128],
                                tp[:],
                            )
                        else:
                            qst = work.tile([128, 4, 128], BF16, tag="qst")
                            nc.scalar.copy(qst[:], tp[:])
                            nc.sync.dma_start(
                                out=qt_dram[
                                    hp * 4 : (hp + 1) * 4, :, tt * 128 : (tt + 1) * 128
                                ].rearrange("h d t -> d h t"),
                                in_=qst[:],
                            )

                prev = None
                for fg in range(3):  # 0=q, 1=k, 2=v
                    # two half-width weight tiles -> next-fg prefetch overlaps
                    wts = []
                    for half in range(2):
                        wt = wq_pool.tile([128, ncl, 512], BF16, tag="wt")
                        # chunked along c so the first matmuls start after the
                        # first quarter has landed
                        for cc in range(4):
                            nc.sync.dma_start(
                                out=wt[:, cc * 4 : (cc + 1) * 4, :],
                                in_=wq[:]
                                .rearrange("(n p) f -> p n f", p=128)[
                                    :,
                                    cc * 4 : (cc + 1) * 4,
                                    fg * HL * D + half * 512 : fg * HL * D
                                    + (half + 1) * 512,
                                ],
                            )
                        wts.append(wt)
                    del wt
                    for tg in range(ng):
                        # x panel + rope tables go through the ACT HWDGE queue,
                        # in parallel with the weight stream on the sync queue
                        xp = x_pool.tile([128, ncl, 512], BF16, tag="xp")
                        for cc in range(4):
                            nc.scalar.dma_start(
                                out=xp[:, cc * 4 : (cc + 1) * 4, :],
                                in_=xt[:]
                                .rearrange("(n p) t -> p n t", p=128)[
                                    :, cc * 4 : (cc + 1) * 4, tg * 512 : (tg + 1) * 512
                                ],
                            )
                        cos_t = sin_t = None
                        if fg < 2:
                            cos_t = cs_pool.tile([128, 4, D], F32, tag="cos")
                            sin_t = cs_pool.tile([128, 4, D], F32, tag="sin")
                            cdram, sdram = (cq, sq) if fg == 0 else (ck, sk)
                            nc.scalar.dma_start(
                                out=cos_t[:],
                                in_=cdram[:].rearrange(
                                    "(g b p) d -> g p b d", b=4, p=128
                                )[tg],
                            )
                            nc.scalar.dma_start(
                                out=sin_t[:],
                                in_=sdram[:].rearrange(
                                    "(g b p) d -> g p b d", b=4, p=128
                                )[tg],
                            )
                        for ts in range(4):
                            tt = tg * 4 + ts
                            ps = qk_ps.tile([128, 2 * 512], F32, tag="qkps")
                            # half-outer order: the first 16 matmuls only need
                            # the first weight half (cheaper fg transitions)
                            for half in range(2):
                                for c in range(ncl):
                                    nc.tensor.matmul(
                                        ps[:, half * 512 : (half + 1) * 512],
                                        lhsT=xp[:, c, ts * 128 : (ts + 1) * 128],
                                        rhs=wts[half][:, c, :],
                                        start=(c == 0),
                                        stop=(c == ncl - 1),
                                    )
                            if prev is not None:
                                phase1_post(*prev)
                            prev = (fg, tt, ps, cos_t, sin_t, ts)
                phase1_post(*prev)

            # ---------- phase 2: attention + output projection ----------
            with (
                tc.tile_pool(name="wp_pool", bufs=1) as wp_pool,
                tc.tile_pool(name="qt_pool", bufs=2) as qt_pool,
                tc.tile_pool(name="pt_pool", bufs=2) as pt_pool,
                tc.tile_pool(name="y_pool", bufs=2) as y_pool,
                tc.tile_pool(name="pa_pool", bufs=2) as pa_pool,
                tc.tile_pool(name="o_pool", bufs=1) as o_pool,
                tc.tile_pool(name="r_pool", bufs=2) as r_pool,
                tc.tile_pool(name="sp_ps", bufs=2, space="PSUM") as sp_ps,
                tc.tile_pool(name="acc_ps", bufs=4, space="PSUM") as acc_ps,
            ):
                def load_qtc(ch):
                    qtc = qt_pool.tile([128, HL, 512], BF16, tag="qtc")
                    nc.sync.dma_start(
                        out=qtc[:],
                        in_=qt_dram[:, :, ch * 512 : (ch + 1) * 512].rearrange(
                            "h d t -> d h t"
                        ),
                    )
                    return qtc

                # first-chunk q tiles before the (large) proj-weight load so
                # the first scores matmuls aren't stuck behind it
                qtc_next = load_qtc(0)
                WPT = wp_pool.tile([128, HL, C], BF16)
                nc.sync.dma_start(out=WPT[:], in_=wp[:].rearrange("h d o -> d h o"))

                def emit_scores(qtc, h, PT, pa):
                    """S^T = K^T.T @ q^T for one head/chunk; exp into PT.
                    GpSimd pair-sums each exp'd pair so the denominator
                    matmul only has to stream half the tiles."""
                    for sg in range(nt // 2):
                        sp = sp_ps.tile([128, 2, 512], F32, tag="sp")
                        for i in range(2):
                            tk = sg * 2 + i
                            nc.tensor.matmul(
                                sp[:, i, :],
                                lhsT=KT[:, h, tk * 128 : (tk + 1) * 128],
                                rhs=qtc[:, h, :],
                                start=True,
                                stop=True,
                            )
                        nc.scalar.activation(
                            PT[:, sg * 2 : sg * 2 + 2, :], sp[:], AF.Exp
                        )
                        nc.gpsimd.tensor_add(
                            pa[:, sg, :], PT[:, sg * 2, :], PT[:, sg * 2 + 1, :]
                        )

                def emit_consume(h, PT, pa, Ysb):
                    """Denominator + PV + normalize for one head/chunk."""
                    ss = acc_ps.tile([1, 512], F32, tag="acc")
                    for i in range(nt // 2):
                        nc.tensor.matmul(
                            ss[:],
                            lhsT=ones_b[:],
                            rhs=pa[:, i, :],
                            start=(i == 0),
                            stop=(i == nt // 2 - 1),
                        )
                    rinv = r_pool.tile([1, 512], F32, tag="rinv")
                    nc.vector.reciprocal(rinv[:], ss[:])
                    yp = acc_ps.tile([128, 512], F32, tag="acc")
                    for i in range(nt):
                        nc.tensor.matmul(
                            yp[:],
                            lhsT=Vs[:, h, i, :],
                            rhs=PT[:, i, :],
                            start=(i == 0),
                            stop=(i == nt - 1),
                        )
                    rbs = r_pool.tile([128, 512], F32, tag="rbs")
                    nc.gpsimd.partition_broadcast(rbs[:], rinv[:])
                    nc.vector.tensor_mul(Ysb[:, h, :], yp[:], rbs[:])

                def emit_proj(ch, Ysb):
                    """Partial output projection for one finished chunk."""
                    for ts in range(4):
                        osb = o_pool.tile([128, C], F32, tag="osb")
                        for ot in range(C // 512):
                            op = acc_ps.tile([128, 512], F32, tag="acc")
                            for h in range(HL):
                                nc.tensor.matmul(
                                    op[:],
                                    lhsT=Ysb[:, h, ts * 128 : (ts + 1) * 128],
                                    rhs=WPT[:, h, ot * 512 : (ot + 1) * 512],
                                    start=(h == 0),
                                    stop=(h == HL - 1),
                                )
                            nc.scalar.copy(osb[:, ot * 512 : (ot + 1) * 512], op[:])
                        trow = ch * 4 + ts
                        nc.sync.dma_start(
                            out=out[:].rearrange("(n p) o -> n p o", p=128)[trow],
                            in_=osb[:],
                        )

                prev = None  # (h, PT, Ysb)
                proj_ready = None  # (ch, Ysb)
                for ch in range(ng):
                    qtc = qtc_next
                    Ysb = y_pool.tile([128, HL, 512], BF16, tag="y")
                    for h in range(HL):
                        PT = pt_pool.tile([128, nt, 512], BF16, tag="pt")
                        pa = pa_pool.tile([128, nt // 2, 512], BF16, tag="pa")
                        emit_scores(qtc, h, PT, pa)
                        if h == 0 and ch + 1 < ng:
                            qtc_next = load_qtc(ch + 1)
                        if prev is not None:
                            emit_consume(*prev)
                        if proj_ready is not None:
                            emit_proj(*proj_ready)
                            proj_ready = None
                        prev = (h, PT, pa, Ysb)
                    proj_ready = (ch, Ysb)
                emit_consume(*prev)
                emit_proj(*proj_ready)

    nc.compile()
    return nc


def prep_inputs(x, cos, sin, w_qkv, w_proj, g_q, g_k, T_=T, b_count=B):
    """Host-side sharding: per-core input dicts."""
    x = np.asarray(x, dtype=np.float32)
    cos = np.asarray(cos, dtype=np.float32)[:T_]
    sin = np.asarray(sin, dtype=np.float32)[:T_]
    w_qkv = np.asarray(w_qkv, dtype=np.float32)
    w_proj = np.asarray(w_proj, dtype=np.float32)
    g_q = np.asarray(g_q, dtype=np.float32)
    g_k = np.asarray(g_k, dtype=np.float32)

    srcidx = np.concatenate([np.arange(64, 128), np.arange(0, 64)])
    sign = np.concatenate([-np.ones(64, np.float32), np.ones(64, np.float32)])
    scale_q = 1.0 / math.sqrt(D)
    cq_np = np.ascontiguousarray(cos * g_q[None, :] * scale_q)
    sq_np = np.ascontiguousarray(sin * sign[None, :] * g_q[srcidx][None, :] * scale_q)
    ck_np = np.ascontiguousarray(cos * g_k[None, :])
    sk_np = np.ascontiguousarray(sin * sign[None, :] * g_k[srcidx][None, :])

    wq_r = w_qkv.reshape(3, H, D, C)
    wp_r = w_proj.reshape(C, H, D)

    in_maps = []
    for core in range(NCORES):
        b = core // 2
        hg = core % 2
        xt_np = np.ascontiguousarray(x[b % b_count][:T_].T).astype(NP_BF16)
        wsel = wq_r[:, hg * HL : (hg + 1) * HL]  # [3, HL, D, C]
        wq_np = np.ascontiguousarray(wsel.reshape(3 * HL * D, C).T).astype(
            NP_BF16
        )  # [C, 3*HL*D]
        wp_np = np.ascontiguousarray(
            wp_r[:, hg * HL : (hg + 1) * HL, :].transpose(1, 2, 0)
        ).astype(NP_BF16)  # [HL, D, C]
        in_maps.append(
            {
                "xt": xt_np,
                "wq": wq_np,
                "wp": wp_np,
                "cq": cq_np,
                "sq": sq_np,
                "ck": ck_np,
                "sk": sk_np,
            }
        )
    return in_maps


_nc_cache = {}


def _get_nc(T_=T):
    if T_ not in _nc_cache:
        _nc_cache[T_] = build(T_)
    return _nc_cache[T_]


def _install_trace_hook():
    """Register the axon NTFF profile hook (missing from this image's antenv)."""
    if "antenv.axon_hooks" in sys.modules:
        return
    try:
        from trn_agent_boot.trn_boot import _ntff_profile_via_ctypes

        hook = _ntff_profile_via_ctypes("/opt/axon/libaxon_pjrt.so")
        mod = types.ModuleType("antenv.axon_hooks")
        mod.get_axon_ntff_profile_hook = lambda: hook
        sys.modules["antenv.axon_hooks"] = mod
    except Exception:
        pass


def run(inputs, T_=T, trace=False, tmpdir=None):
    """Run the sharded kernel; returns (full output [B, T, C] fp32, results obj)."""
    nc = _get_nc(T_)
    in_maps = prep_inputs(**inputs, T_=T_)
    kwargs = {}
    if trace:
        _install_trace_hook()
        kwargs = dict(trace=True, tmpdir=tmpdir)
    res = run_bass_kernel_spmd(nc, in_maps, core_ids=list(range(NCORES)), **kwargs)
    outs = [res.results[i]["out"] for i in range(NCORES)]
    full = np.stack([outs[2 * b] + outs[2 * b + 1] for b in range(B)], axis=0).astype(
        np.float32
    )
    return full, res


def kernel(x, cos, sin, w_qkv, w_proj, g_q, g_k):
    full, _ = run(
        dict(x=x, cos=cos, sin=sin, w_qkv=w_qkv, w_proj=w_proj, g_q=g_q, g_k=g_k)
    )
    return full


# revision 21
# speedup vs baseline: 1.4275x; 1.4275x over previous
"""Fused multi-head attention block (QKV proj + per-head RMSNorm + RoPE +
softmax attention + output proj) on 8 Trainium2 NeuronCores.

Sharding: core c handles (batch b = c//2, head-group hg = c%2 of 8 heads).
Each core computes a partial output projection over its 8 heads; the host
sums the two partials per batch.

Self-contained: hardcodes B=4, T=2048, C=2048, H=16, D=128.
"""

import math
import sys
import types

import numpy as np
import ml_dtypes

import concourse.bass as bass
import concourse.bacc as bacc
import concourse.tile as tile
from concourse import mybir
from concourse.bass_utils import run_bass_kernel_spmd
from concourse.masks import make_identity

BF16 = mybir.dt.bfloat16
F32 = mybir.dt.float32
NP_BF16 = ml_dtypes.bfloat16
AF = mybir.ActivationFunctionType
ALU = mybir.AluOpType
AX = mybir.AxisListType

B, T, C, H, D = 4, 2048, 2048, 16, 128
HL = H // 2  # heads per core
EPS = 1e-6
NCORES = 8


def _bcast_mid(ap2d, n):
    """[P, F] AP -> [P, n, F] AP broadcast along a new middle dim (step 0)."""
    return bass.AP(
        tensor=ap2d.tensor,
        offset=ap2d.offset,
        ap=[ap2d.ap[0], [0, n], ap2d.ap[1]],
    )


def build(T_=T):
    """Build + compile the per-core Bass program (identical on all cores)."""
    nt = T_ // 128  # number of 128-row t-tiles
    ng = T_ // 512  # number of 512-row t-groups (tq chunks)
    ncl = C // 128  # contraction tiles over C

    nc = bacc.Bacc("TRN2", target_bir_lowering=False, debug=False, num_devices=NCORES)

    xt = nc.dram_tensor("xt", [C, T_], BF16, kind="ExternalInput")  # x[b].T
    wq = nc.dram_tensor("wq", [C, 3 * HL * D], BF16, kind="ExternalInput")  # (c, f')
    wp = nc.dram_tensor("wp", [HL, D, C], BF16, kind="ExternalInput")  # (h, dv, o)
    cq = nc.dram_tensor("cq", [T_, D], F32, kind="ExternalInput")
    sq = nc.dram_tensor("sq", [T_, D], F32, kind="ExternalInput")
    ck = nc.dram_tensor("ck", [T_, D], F32, kind="ExternalInput")
    sk = nc.dram_tensor("sk", [T_, D], F32, kind="ExternalInput")
    out = nc.dram_tensor("out", [T_, C], F32, kind="ExternalOutput")

    with tile.TileContext(nc) as tc:
        with (
            tc.tile_pool(name="persist", bufs=1) as persist,
            tc.tile_pool(name="dram", bufs=1, space="DRAM") as dpool,
        ):
            ident = persist.tile([128, 128], BF16)
            make_identity(nc, ident[:])
            ones_b = persist.tile([128, 1], BF16)
            nc.vector.memset(ones_b[:], 1.0)
            ones_f = persist.tile([1, 128], F32)
            nc.vector.memset(ones_f[:], 1.0)

            KT = persist.tile([128, HL, T_], BF16)  # k^T: (d, h, t)
            Vs = persist.tile([128, HL, nt, D], BF16)  # v: (t-part, h, t-tile, dv)
            qt_dram = dpool.tile([HL, D, T_], BF16)  # q^T spill: (h, d, t)

            # ---------- phase 1: QKV proj + RMS norm + RoPE + transposes ----------
            with (
                tc.tile_pool(name="wq_pool", bufs=3) as wq_pool,
                tc.tile_pool(name="x_pool", bufs=2) as x_pool,
                tc.tile_pool(name="cs_pool", bufs=2) as cs_pool,
                tc.tile_pool(name="work", bufs=2) as work,
                tc.tile_pool(name="qk_ps", bufs=2, space="PSUM") as qk_ps,
                tc.tile_pool(name="tp_ps", bufs=2, space="PSUM") as tp_ps,
            ):

                def phase1_post(fg, tt, ps, cos_t, sin_t, ts):
                    """Consume a finished QKV psum tile: norm+rope+transpose
                    (q/k) or copy out (v)."""
                    ps3 = ps[:].rearrange("p (h d) -> p h d", h=HL)
                    if fg == 2:
                        nc.scalar.copy(Vs[:, :, tt, :], ps3)
                        return
                    # RMS norm stats (Square on ACT: DVE can't read two PSUM
                    # operands)
                    sqs = work.tile([128, HL * D], F32, tag="sqs")
                    nc.scalar.activation(sqs[:], ps[:], AF.Square)
                    sums = work.tile([128, HL], F32, tag="sums")
                    nc.vector.tensor_reduce(
                        out=sums[:],
                        in_=sqs[:].rearrange("p (h d) -> p h d", h=HL),
                        axis=AX.X,
                        op=ALU.add,
                    )
                    ms = work.tile([128, HL], F32, tag="ms")
                    nc.vector.tensor_scalar(
                        out=ms[:],
                        in0=sums[:],
                        scalar1=1.0 / D,
                        scalar2=EPS,
                        op0=ALU.mult,
                        op1=ALU.add,
                    )
                    srt = work.tile([128, HL], F32, tag="srt")
                    nc.scalar.sqrt(srt[:], ms[:])
                    rstd = work.tile([128, HL], F32, tag="rstd")
                    nc.vector.reciprocal(rstd[:], srt[:])
                    # RoPE (scale/sign/g folded into cos/sin host-side)
                    cs_ = cos_t[:, ts, :]
                    sn_ = sin_t[:, ts, :]
                    t3a = work.tile([128, HL, D], F32, tag="t3a")
                    nc.vector.tensor_mul(t3a[:], ps3, _bcast_mid(cs_, HL))
                    t3b = work.tile([128, HL, D], F32, tag="t3b")
                    nc.vector.tensor_mul(
                        t3b[:, :, 0:64],
                        ps3[:, :, 64:128],
                        _bcast_mid(sn_[:, 0:64], HL),
                    )
                    nc.vector.tensor_mul(
                        t3b[:, :, 64:128],
                        ps3[:, :, 0:64],
                        _bcast_mid(sn_[:, 64:128], HL),
                    )
                    t3 = work.tile([128, HL, D], F32, tag="t3")
                    nc.vector.tensor_add(t3[:], t3a[:], t3b[:])
                    rbf = work.tile([128, HL, D], BF16, tag="rbf")
                    for h in range(HL):
                        nc.vector.tensor_scalar_mul(
                            rbf[:, h, :], t3[:, h, :], rstd[:, h : h + 1]
                        )
                    # transpose to (d, t) in 2 groups of 4 heads
                    for hp in range(2):
                        tp = tp_ps.tile([128, 4, 128], BF16, tag="tp")
                        for j in range(4):
                            nc.tensor.transpose(
                                tp[:, j, :], rbf[:, hp * 4 + j, :], ident[:]
                            )
                        if fg == 1:
                            nc.scalar.copy(
                                KT[:, hp * 4 : (hp + 1) * 4, tt * 128 : (tt + 1) * 128],
                                tp[:],
                            )
                        else:
                            qst = work.tile([128, 4, 128], BF16, tag="qst")
                            nc.scalar.copy(qst[:], tp[:])
                            nc.sync.dma_start(
                                out=qt_dram[
                                    hp * 4 : (hp + 1) * 4, :, tt * 128 : (tt + 1) * 128
                                ].rearrange("h d t -> d h t"),
                                in_=qst[:],
                            )

                prev = None
                for fg in range(3):  # 0=q, 1=k, 2=v
                    # two half-width weight tiles -> next-fg prefetch overlaps
                    wts = []
                    for half in range(2):
                        wt = wq_pool.tile([128, ncl, 512], BF16, tag="wt")
                        # chunked along c so the first matmuls start after the
                        # first quarter has landed
                        for cc in range(4):
                            nc.sync.dma_start(
                                out=wt[:, cc * 4 : (cc + 1) * 4, :],
                                in_=wq[:]
                                .rearrange("(n p) f -> p n f", p=128)[
                                    :,
                                    cc * 4 : (cc + 1) * 4,
                                    fg * HL * D + half * 512 : fg * HL * D
                                    + (half + 1) * 512,
                                ],
                            )
                        wts.append(wt)
                    del wt
                    for tg in range(ng):
                        # x panel + rope tables go through the ACT HWDGE queue,
                        # in parallel with the weight stream on the sync queue
                        xp = x_pool.tile([128, ncl, 512], BF16, tag="xp")
                        for cc in range(4):
                            nc.scalar.dma_start(
                                out=xp[:, cc * 4 : (cc + 1) * 4, :],
                                in_=xt[:]
                                .rearrange("(n p) t -> p n t", p=128)[
                                    :, cc * 4 : (cc + 1) * 4, tg * 512 : (tg + 1) * 512
                                ],
                            )
                        cos_t = sin_t = None
                        if fg < 2:
                            cos_t = cs_pool.tile([128, 4, D], F32, tag="cos")
                            sin_t = cs_pool.tile([128, 4, D], F32, tag="sin")
                            cdram, sdram = (cq, sq) if fg == 0 else (ck, sk)
                            nc.scalar.dma_start(
                                out=cos_t[:],
                                in_=cdram[:].rearrange(
                                    "(g b p) d -> g p b d", b=4, p=128
                                )[tg],
                            )
                            nc.scalar.dma_start(
                                out=sin_t[:],
                                in_=sdram[:].rearrange(
                                    "(g b p) d -> g p b d", b=4, p=128
                                )[tg],
                            )
                        for ts in range(4):
                            tt = tg * 4 + ts
                            ps = qk_ps.tile([128, 2 * 512], F32, tag="qkps")
                            # half-outer order: the first 16 matmuls only need
                            # the first weight half (cheaper fg transitions)
                            for half in range(2):
                                for c in range(ncl):
                                    nc.tensor.matmul(
                                        ps[:, half * 512 : (half + 1) * 512],
                                        lhsT=xp[:, c, ts * 128 : (ts + 1) * 128],
                                        rhs=wts[half][:, c, :],
                                        start=(c == 0),
                                        stop=(c == ncl - 1),
                                    )
                            if prev is not None:
                                phase1_post(*prev)
                            prev = (fg, tt, ps, cos_t, sin_t, ts)
                phase1_post(*prev)

            # ---------- phase 2: attention + output projection ----------
            with (
                tc.tile_pool(name="wp_pool", bufs=1) as wp_pool,
                tc.tile_pool(name="qt_pool", bufs=2) as qt_pool,
                tc.tile_pool(name="pt_pool", bufs=2) as pt_pool,
                tc.tile_pool(name="y_pool", bufs=2) as y_pool,
                tc.tile_pool(name="o_pool", bufs=2) as o_pool,
                tc.tile_pool(name="r_pool", bufs=2) as r_pool,
                tc.tile_pool(name="sp_ps", bufs=2, space="PSUM") as sp_ps,
                tc.tile_pool(name="acc_ps", bufs=4, space="PSUM") as acc_ps,
            ):
                def load_qtc(ch):
                    qtc = qt_pool.tile([128, HL, 512], BF16, tag="qtc")
                    nc.sync.dma_start(
                        out=qtc[:],
                        in_=qt_dram[:, :, ch * 512 : (ch + 1) * 512].rearrange(
                            "h d t -> d h t"
                        ),
                    )
                    return qtc

                # first-chunk q tiles before the (large) proj-weight load so
                # the first scores matmuls aren't stuck behind it
                qtc_next = load_qtc(0)
                WPT = wp_pool.tile([128, HL, C], BF16)
                nc.sync.dma_start(out=WPT[:], in_=wp[:].rearrange("h d o -> d h o"))

                def emit_scores(qtc, h, PT):
                    """S^T = K^T.T @ q^T for one head/chunk; exp into PT."""
                    for sg in range(nt // 2):
                        sp = sp_ps.tile([128, 2, 512], F32, tag="sp")
                        for i in range(2):
                            tk = sg * 2 + i
                            nc.tensor.matmul(
                                sp[:, i, :],
                                lhsT=KT[:, h, tk * 128 : (tk + 1) * 128],
                                rhs=qtc[:, h, :],
                                start=True,
                                stop=True,
                            )
                        nc.scalar.activation(
                            PT[:, sg * 2 : sg * 2 + 2, :], sp[:], AF.Exp
                        )

                def emit_consume(h, PT, Ysb):
                    """Denominator + PV + normalize for one head/chunk."""
                    ss = acc_ps.tile([1, 512], F32, tag="acc")
                    for i in range(nt):
                        nc.tensor.matmul(
                            ss[:],
                            lhsT=ones_b[:],
                            rhs=PT[:, i, :],
                            start=(i == 0),
                            stop=(i == nt - 1),
                        )
                    rinv = r_pool.tile([1, 512], F32, tag="rinv")
                    nc.vector.reciprocal(rinv[:], ss[:])
                    yp = acc_ps.tile([128, 512], F32, tag="acc")
                    for i in range(nt):
                        nc.tensor.matmul(
                            yp[:],
                            lhsT=Vs[:, h, i, :],
                            rhs=PT[:, i, :],
                            start=(i == 0),
                            stop=(i == nt - 1),
                        )
                    rbs = r_pool.tile([128, 512], F32, tag="rbs")
                    nc.gpsimd.partition_broadcast(rbs[:], rinv[:])
                    nc.vector.tensor_mul(Ysb[:, h, :], yp[:], rbs[:])

                def emit_proj(ch, Ysb):
                    """Partial output projection for one finished chunk."""
                    for ts in range(4):
                        osb = o_pool.tile([128, C], F32, tag="osb")
                        for ot in range(C // 512):
                            op = acc_ps.tile([128, 512], F32, tag="acc")
                            for h in range(HL):
                                nc.tensor.matmul(
                                    op[:],
                                    lhsT=Ysb[:, h, ts * 128 : (ts + 1) * 128],
                                    rhs=WPT[:, h, ot * 512 : (ot + 1) * 512],
                                    start=(h == 0),
                                    stop=(h == HL - 1),
                                )
                            nc.scalar.copy(osb[:, ot * 512 : (ot + 1) * 512], op[:])
                        trow = ch * 4 + ts
                        nc.sync.dma_start(
                            out=out[:].rearrange("(n p) o -> n p o", p=128)[trow],
                            in_=osb[:],
                        )

                prev = None  # (h, PT, Ysb)
                proj_ready = None  # (ch, Ysb)
                for ch in range(ng):
                    qtc = qtc_next
                    Ysb = y_pool.tile([128, HL, 512], BF16, tag="y")
                    for h in range(HL):
                        PT = pt_pool.tile([128, nt, 512], BF16, tag="pt")
                        emit_scores(qtc, h, PT)
                        if h == 0 and ch + 1 < ng:
                            qtc_next = load_qtc(ch + 1)
                        if prev is not None:
                            emit_consume(*prev)
                        if proj_ready is not None:
                            emit_proj(*proj_ready)
                            proj_ready = None
                        prev = (h, PT, Ysb)
                    proj_ready = (ch, Ysb)
                emit_consume(*prev)
                emit_proj(*proj_ready)

    nc.compile()
    return nc


def prep_inputs(x, cos, sin, w_qkv, w_proj, g_q, g_k, T_=T, b_count=B):
    """Host-side sharding: per-core input dicts."""
    x = np.asarray(x, dtype=np.float32)
    cos = np.asarray(cos, dtype=np.float32)[:T_]
    sin = np.asarray(sin, dtype=np.float32)[:T_]
    w_qkv = np.asarray(w_qkv, dtype=np.float32)
    w_proj = np.asarray(w_proj, dtype=np.float32)
    g_q = np.asarray(g_q, dtype=np.float32)
    g_k = np.asarray(g_k, dtype=np.float32)

    srcidx = np.concatenate([np.arange(64, 128), np.arange(0, 64)])
    sign = np.concatenate([-np.ones(64, np.float32), np.ones(64, np.float32)])
    scale_q = 1.0 / math.sqrt(D)
    cq_np = np.ascontiguousarray(cos * g_q[None, :] * scale_q)
    sq_np = np.ascontiguousarray(sin * sign[None, :] * g_q[srcidx][None, :] * scale_q)
    ck_np = np.ascontiguousarray(cos * g_k[None, :])
    sk_np = np.ascontiguousarray(sin * sign[None, :] * g_k[srcidx][None, :])

    wq_r = w_qkv.reshape(3, H, D, C)
    wp_r = w_proj.reshape(C, H, D)

    in_maps = []
    for core in range(NCORES):
        b = core // 2
        hg = core % 2
        xt_np = np.ascontiguousarray(x[b % b_count][:T_].T).astype(NP_BF16)
        wsel = wq_r[:, hg * HL : (hg + 1) * HL]  # [3, HL, D, C]
        wq_np = np.ascontiguousarray(wsel.reshape(3 * HL * D, C).T).astype(
            NP_BF16
        )  # [C, 3*HL*D]
        wp_np = np.ascontiguousarray(
            wp_r[:, hg * HL : (hg + 1) * HL, :].transpose(1, 2, 0)
        ).astype(NP_BF16)  # [HL, D, C]
        in_maps.append(
            {
                "xt": xt_np,
                "wq": wq_np,
                "wp": wp_np,
                "cq": cq_np,
                "sq": sq_np,
                "ck": ck_np,
                "sk": sk_np,
            }
        )
    return in_maps


_nc_cache = {}


def _get_nc(T_=T):
    if T_ not in _nc_cache:
        _nc_cache[T_] = build(T_)
    return _nc_cache[T_]


def _install_trace_hook():
    """Register the axon NTFF profile hook (missing from this image's antenv)."""
    if "antenv.axon_hooks" in sys.modules:
        return
    try:
        from trn_agent_boot.trn_boot import _ntff_profile_via_ctypes

        hook = _ntff_profile_via_ctypes("/opt/axon/libaxon_pjrt.so")
        mod = types.ModuleType("antenv.axon_hooks")
        mod.get_axon_ntff_profile_hook = lambda: hook
        sys.modules["antenv.axon_hooks"] = mod
    except Exception:
        pass


def run(inputs, T_=T, trace=False, tmpdir=None):
    """Run the sharded kernel; returns (full output [B, T, C] fp32, results obj)."""
    nc = _get_nc(T_)
    in_maps = prep_inputs(**inputs, T_=T_)
    kwargs = {}
    if trace:
        _install_trace_hook()
        kwargs = dict(trace=True, tmpdir=tmpdir)
    res = run_bass_kernel_spmd(nc, in_maps, core_ids=list(range(NCORES)), **kwargs)
    outs = [res.results[i]["out"] for i in range(NCORES)]
    full = np.stack([outs[2 * b] + outs[2 * b + 1] for b in range(B)], axis=0).astype(
        np.float32
    )
    return full, res


def kernel(x, cos, sin, w_qkv, w_proj, g_q, g_k):
    full, _ = run(
        dict(x=x, cos=cos, sin=sin, w_qkv=w_qkv, w_proj=w_proj, g_q=g_q, g_k=g_k)
    )
    return full


# revision 22
# speedup vs baseline: 1.4846x; 1.0400x over previous
"""Fused multi-head attention block (QKV proj + per-head RMSNorm + RoPE +
softmax attention + output proj) on 8 Trainium2 NeuronCores.

Sharding: core c handles (batch b = c//2, head-group hg = c%2 of 8 heads).
Each core computes a partial output projection over its 8 heads; the host
sums the two partials per batch.

Self-contained: hardcodes B=4, T=2048, C=2048, H=16, D=128.
"""

import math
import sys
import types

import numpy as np
import ml_dtypes

import concourse.bass as bass
import concourse.bacc as bacc
import concourse.tile as tile
from concourse import mybir
from concourse.bass_utils import run_bass_kernel_spmd
from concourse.masks import make_identity

BF16 = mybir.dt.bfloat16
F32 = mybir.dt.float32
NP_BF16 = ml_dtypes.bfloat16
AF = mybir.ActivationFunctionType
ALU = mybir.AluOpType
AX = mybir.AxisListType

B, T, C, H, D = 4, 2048, 2048, 16, 128
HL = H // 2  # heads per core
EPS = 1e-6
NCORES = 8


def _bcast_mid(ap2d, n):
    """[P, F] AP -> [P, n, F] AP broadcast along a new middle dim (step 0)."""
    return bass.AP(
        tensor=ap2d.tensor,
        offset=ap2d.offset,
        ap=[ap2d.ap[0], [0, n], ap2d.ap[1]],
    )


def build(T_=T):
    """Build + compile the per-core Bass program (identical on all cores)."""
    nt = T_ // 128  # number of 128-row t-tiles
    ng = T_ // 512  # number of 512-row t-groups (tq chunks)
    ncl = C // 128  # contraction tiles over C

    nc = bacc.Bacc("TRN2", target_bir_lowering=False, debug=False, num_devices=NCORES)

    xt = nc.dram_tensor("xt", [C, T_], BF16, kind="ExternalInput")  # x[b].T
    wq = nc.dram_tensor("wq", [C, 3 * HL * D], BF16, kind="ExternalInput")  # (c, f')
    wp = nc.dram_tensor("wp", [HL, D, C], BF16, kind="ExternalInput")  # (h, dv, o)
    cq = nc.dram_tensor("cq", [T_, D], F32, kind="ExternalInput")
    sq = nc.dram_tensor("sq", [T_, D], F32, kind="ExternalInput")
    ck = nc.dram_tensor("ck", [T_, D], F32, kind="ExternalInput")
    sk = nc.dram_tensor("sk", [T_, D], F32, kind="ExternalInput")
    out = nc.dram_tensor("out", [T_, C], F32, kind="ExternalOutput")

    with tile.TileContext(nc) as tc:
        with (
            tc.tile_pool(name="persist", bufs=1) as persist,
            tc.tile_pool(name="dram", bufs=1, space="DRAM") as dpool,
        ):
            ident = persist.tile([128, 128], BF16)
            make_identity(nc, ident[:])
            ones_b = persist.tile([128, 1], BF16)
            nc.vector.memset(ones_b[:], 1.0)
            ones_f = persist.tile([1, 128], F32)
            nc.vector.memset(ones_f[:], 1.0)

            KT = persist.tile([128, HL, T_], BF16)  # k^T: (d, h, t)
            Vs = persist.tile([128, HL, nt, D], BF16)  # v: (t-part, h, t-tile, dv)
            qt_dram = dpool.tile([HL, D, T_], BF16)  # q^T spill: (h, d, t)

            # ---------- phase 1: QKV proj + RMS norm + RoPE + transposes ----------
            with (
                tc.tile_pool(name="wq_pool", bufs=3) as wq_pool,
                tc.tile_pool(name="x_pool", bufs=2) as x_pool,
                tc.tile_pool(name="cs_pool", bufs=2) as cs_pool,
                tc.tile_pool(name="work", bufs=2) as work,
                tc.tile_pool(name="qk_ps", bufs=2, space="PSUM") as qk_ps,
                tc.tile_pool(name="tp_ps", bufs=2, space="PSUM") as tp_ps,
            ):

                def phase1_post(fg, tt, ps, cos_t, sin_t, ts):
                    """Consume a finished QKV psum tile: norm+rope+transpose
                    (q/k) or copy out (v)."""
                    ps3 = ps[:].rearrange("p (h d) -> p h d", h=HL)
                    if fg == 2:
                        nc.scalar.copy(Vs[:, :, tt, :], ps3)
                        return
                    # RMS norm stats (Square on ACT: DVE can't read two PSUM
                    # operands)
                    sqs = work.tile([128, HL * D], F32, tag="sqs")
                    nc.scalar.activation(sqs[:], ps[:], AF.Square)
                    sums = work.tile([128, HL], F32, tag="sums")
                    nc.vector.tensor_reduce(
                        out=sums[:],
                        in_=sqs[:].rearrange("p (h d) -> p h d", h=HL),
                        axis=AX.X,
                        op=ALU.add,
                    )
                    ms = work.tile([128, HL], F32, tag="ms")
                    nc.vector.tensor_scalar(
                        out=ms[:],
                        in0=sums[:],
                        scalar1=1.0 / D,
                        scalar2=EPS,
                        op0=ALU.mult,
                        op1=ALU.add,
                    )
                    srt = work.tile([128, HL], F32, tag="srt")
                    nc.scalar.sqrt(srt[:], ms[:])
                    rstd = work.tile([128, HL], F32, tag="rstd")
                    nc.vector.reciprocal(rstd[:], srt[:])
                    # RoPE (scale/sign/g folded into cos/sin host-side)
                    cs_ = cos_t[:, ts, :]
                    sn_ = sin_t[:, ts, :]
                    t3a = work.tile([128, HL, D], F32, tag="t3a")
                    nc.vector.tensor_mul(t3a[:], ps3, _bcast_mid(cs_, HL))
                    t3b = work.tile([128, HL, D], F32, tag="t3b")
                    nc.vector.tensor_mul(
                        t3b[:, :, 0:64],
                        ps3[:, :, 64:128],
                        _bcast_mid(sn_[:, 0:64], HL),
                    )
                    nc.vector.tensor_mul(
                        t3b[:, :, 64:128],
                        ps3[:, :, 0:64],
                        _bcast_mid(sn_[:, 64:128], HL),
                    )
                    t3 = work.tile([128, HL, D], F32, tag="t3")
                    nc.vector.tensor_add(t3[:], t3a[:], t3b[:])
                    rbf = work.tile([128, HL, D], BF16, tag="rbf")
                    for h in range(HL):
                        nc.vector.tensor_scalar_mul(
                            rbf[:, h, :], t3[:, h, :], rstd[:, h : h + 1]
                        )
                    # transpose to (d, t) in 2 groups of 4 heads
                    for hp in range(2):
                        tp = tp_ps.tile([128, 4, 128], BF16, tag="tp")
                        for j in range(4):
                            nc.tensor.transpose(
                                tp[:, j, :], rbf[:, hp * 4 + j, :], ident[:]
                            )
                        if fg == 1:
                            nc.scalar.copy(
                                KT[:, hp * 4 : (hp + 1) * 4, tt * 128 : (tt + 1) * 128],
                                tp[:],
                            )
                        else:
                            qst = work.tile([128, 4, 128], BF16, tag="qst")
                            nc.scalar.copy(qst[:], tp[:])
                            nc.sync.dma_start(
                                out=qt_dram[
                                    hp * 4 : (hp + 1) * 4, :, tt * 128 : (tt + 1) * 128
                                ].rearrange("h d t -> d h t"),
                                in_=qst[:],
                            )

                prev = None
                for fg in range(3):  # 0=q, 1=k, 2=v
                    # two half-width weight tiles -> next-fg prefetch overlaps
                    wts = []
                    for half in range(2):
                        wt = wq_pool.tile([128, ncl, 512], BF16, tag="wt")
                        # chunked along c so the first matmuls start after the
                        # first quarter has landed
                        for cc in range(4):
                            nc.sync.dma_start(
                                out=wt[:, cc * 4 : (cc + 1) * 4, :],
                                in_=wq[:]
                                .rearrange("(n p) f -> p n f", p=128)[
                                    :,
                                    cc * 4 : (cc + 1) * 4,
                                    fg * HL * D + half * 512 : fg * HL * D
                                    + (half + 1) * 512,
                                ],
                            )
                        wts.append(wt)
                    del wt
                    for tg in range(ng):
                        # x panel + rope tables go through the ACT HWDGE queue,
                        # in parallel with the weight stream on the sync queue
                        xp = x_pool.tile([128, ncl, 512], BF16, tag="xp")
                        for cc in range(4):
                            nc.scalar.dma_start(
                                out=xp[:, cc * 4 : (cc + 1) * 4, :],
                                in_=xt[:]
                                .rearrange("(n p) t -> p n t", p=128)[
                                    :, cc * 4 : (cc + 1) * 4, tg * 512 : (tg + 1) * 512
                                ],
                            )
                        cos_t = sin_t = None
                        if fg < 2:
                            cos_t = cs_pool.tile([128, 4, D], F32, tag="cos")
                            sin_t = cs_pool.tile([128, 4, D], F32, tag="sin")
                            cdram, sdram = (cq, sq) if fg == 0 else (ck, sk)
                            nc.scalar.dma_start(
                                out=cos_t[:],
                                in_=cdram[:].rearrange(
                                    "(g b p) d -> g p b d", b=4, p=128
                                )[tg],
                            )
                            nc.scalar.dma_start(
                                out=sin_t[:],
                                in_=sdram[:].rearrange(
                                    "(g b p) d -> g p b d", b=4, p=128
                                )[tg],
                            )
                        for ts in range(4):
                            tt = tg * 4 + ts
                            ps = qk_ps.tile([128, 2 * 512], F32, tag="qkps")
                            # half-outer order: the first 16 matmuls only need
                            # the first weight half (cheaper fg transitions)
                            for half in range(2):
                                for c in range(ncl):
                                    nc.tensor.matmul(
                                        ps[:, half * 512 : (half + 1) * 512],
                                        lhsT=xp[:, c, ts * 128 : (ts + 1) * 128],
                                        rhs=wts[half][:, c, :],
                                        start=(c == 0),
                                        stop=(c == ncl - 1),
                                    )
                            if prev is not None:
                                phase1_post(*prev)
                            prev = (fg, tt, ps, cos_t, sin_t, ts)
                phase1_post(*prev)

            # ---------- phase 2: attention + output projection ----------
            with (
                tc.tile_pool(name="wp_pool", bufs=1) as wp_pool,
                tc.tile_pool(name="qt_pool", bufs=2) as qt_pool,
                tc.tile_pool(name="pt_pool", bufs=2) as pt_pool,
                tc.tile_pool(name="y_pool", bufs=2) as y_pool,
                tc.tile_pool(name="pa_pool", bufs=2) as pa_pool,
                tc.tile_pool(name="o_pool", bufs=1) as o_pool,
                tc.tile_pool(name="r_pool", bufs=2) as r_pool,
                tc.tile_pool(name="sp_ps", bufs=2, space="PSUM") as sp_ps,
                tc.tile_pool(name="acc_ps", bufs=4, space="PSUM") as acc_ps,
            ):
                def load_qtc(ch):
                    qtc = qt_pool.tile([128, HL, 512], BF16, tag="qtc")
                    nc.sync.dma_start(
                        out=qtc[:],
                        in_=qt_dram[:, :, ch * 512 : (ch + 1) * 512].rearrange(
                            "h d t -> d h t"
                        ),
                    )
                    return qtc

                # first-chunk q tiles before the (large) proj-weight load so
                # the first scores matmuls aren't stuck behind it
                qtc_next = load_qtc(0)
                WPT = wp_pool.tile([128, HL, C], BF16)
                nc.sync.dma_start(out=WPT[:], in_=wp[:].rearrange("h d o -> d h o"))

                def emit_scores(qtc, h, PT, pa):
                    """S^T = K^T.T @ q^T for one head/chunk; exp into PT."""
                    for sg in range(nt // 2):
                        sp = sp_ps.tile([128, 2, 512], F32, tag="sp")
                        for i in range(2):
                            tk = sg * 2 + i
                            nc.tensor.matmul(
                                sp[:, i, :],
                                lhsT=KT[:, h, tk * 128 : (tk + 1) * 128],
                                rhs=qtc[:, h, :],
                                start=True,
                                stop=True,
                            )
                        nc.scalar.activation(
                            PT[:, sg * 2 : sg * 2 + 2, :], sp[:], AF.Exp
                        )
                        nc.vector.tensor_add(
                            pa[:, sg, :], PT[:, sg * 2, :], PT[:, sg * 2 + 1, :]
                        )

                def emit_consume(h, PT, pa, Ysb):
                    """Denominator + PV + normalize for one head/chunk."""
                    ss = acc_ps.tile([1, 512], F32, tag="acc")
                    for i in range(nt // 2):
                        nc.tensor.matmul(
                            ss[:],
                            lhsT=ones_b[:],
                            rhs=pa[:, i, :],
                            start=(i == 0),
                            stop=(i == nt // 2 - 1),
                        )
                    rinv = r_pool.tile([1, 512], F32, tag="rinv")
                    nc.vector.reciprocal(rinv[:], ss[:])
                    yp = acc_ps.tile([128, 512], F32, tag="acc")
                    for i in range(nt):
                        nc.tensor.matmul(
                            yp[:],
                            lhsT=Vs[:, h, i, :],
                            rhs=PT[:, i, :],
                            start=(i == 0),
                            stop=(i == nt - 1),
                        )
                    rbs = r_pool.tile([128, 512], F32, tag="rbs")
                    nc.gpsimd.partition_broadcast(rbs[:], rinv[:])
                    nc.vector.tensor_mul(Ysb[:, h, :], yp[:], rbs[:])

                def emit_proj(ch, Ysb):
                    """Partial output projection for one finished chunk."""
                    for ts in range(4):
                        osb = o_pool.tile([128, C], F32, tag="osb")
                        for ot in range(C // 512):
                            op = acc_ps.tile([128, 512], F32, tag="acc")
                            for h in range(HL):
                                nc.tensor.matmul(
                                    op[:],
                                    lhsT=Ysb[:, h, ts * 128 : (ts + 1) * 128],
                                    rhs=WPT[:, h, ot * 512 : (ot + 1) * 512],
                                    start=(h == 0),
                                    stop=(h == HL - 1),
                                )
                            nc.scalar.copy(osb[:, ot * 512 : (ot + 1) * 512], op[:])
                        trow = ch * 4 + ts
                        nc.sync.dma_start(
                            out=out[:].rearrange("(n p) o -> n p o", p=128)[trow],
                            in_=osb[:],
                        )

                prev = None  # (h, PT, Ysb)
                proj_ready = None  # (ch, Ysb)
                for ch in range(ng):
                    qtc = qtc_next
                    Ysb = y_pool.tile([128, HL, 512], BF16, tag="y")
                    for h in range(HL):
                        PT = pt_pool.tile([128, nt, 512], BF16, tag="pt")
                        pa = pa_pool.tile([128, nt // 2, 512], BF16, tag="pa")
                        emit_scores(qtc, h, PT, pa)
                        if h == 0 and ch + 1 < ng:
                            qtc_next = load_qtc(ch + 1)
                        if prev is not None:
                            emit_consume(*prev)
                        if proj_ready is not None:
                            emit_proj(*proj_ready)
                            proj_ready = None
                        prev = (h, PT, pa, Ysb)
                    proj_ready = (ch, Ysb)
                emit_consume(*prev)
                emit_proj(*proj_ready)

    nc.compile()
    return nc


def prep_inputs(x, cos, sin, w_qkv, w_proj, g_q, g_k, T_=T, b_count=B):
    """Host-side sharding: per-core input dicts."""
    x = np.asarray(x, dtype=np.float32)
    cos = np.asarray(cos, dtype=np.float32)[:T_]
    sin = np.asarray(sin, dtype=np.float32)[:T_]
    w_qkv = np.asarray(w_qkv, dtype=np.float32)
    w_proj = np.asarray(w_proj, dtype=np.float32)
    g_q = np.asarray(g_q, dtype=np.float32)
    g_k = np.asarray(g_k, dtype=np.float32)

    srcidx = np.concatenate([np.arange(64, 128), np.arange(0, 64)])
    sign = np.concatenate([-np.ones(64, np.float32), np.ones(64, np.float32)])
    scale_q = 1.0 / math.sqrt(D)
    cq_np = np.ascontiguousarray(cos * g_q[None, :] * scale_q)
    sq_np = np.ascontiguousarray(sin * sign[None, :] * g_q[srcidx][None, :] * scale_q)
    ck_np = np.ascontiguousarray(cos * g_k[None, :])
    sk_np = np.ascontiguousarray(sin * sign[None, :] * g_k[srcidx][None, :])

    wq_r = w_qkv.reshape(3, H, D, C)
    wp_r = w_proj.reshape(C, H, D)

    in_maps = []
    for core in range(NCORES):
        b = core // 2
        hg = core % 2
        xt_np = np.ascontiguousarray(x[b % b_count][:T_].T).astype(NP_BF16)
        wsel = wq_r[:, hg * HL : (hg + 1) * HL]  # [3, HL, D, C]
        wq_np = np.ascontiguousarray(wsel.reshape(3 * HL * D, C).T).astype(
            NP_BF16
        )  # [C, 3*HL*D]
        wp_np = np.ascontiguousarray(
            wp_r[:, hg * HL : (hg + 1) * HL, :].transpose(1, 2, 0)
        ).astype(NP_BF16)  # [HL, D, C]
        in_maps.append(
            {
                "xt": xt_np,
                "wq": wq_np,
                "wp": wp_np,
                "cq": cq_np,
                "sq": sq_np,
                "ck": ck_np,
                "sk": sk_np,
            }
        )
    return in_maps


_nc_cache = {}


def _get_nc(T_=T):
    if T_ not in _nc_cache:
        _nc_cache[T_] = build(T_)
    return _nc_cache[T_]


def _install_trace_hook():
    """Register the axon NTFF profile hook (missing from this image's antenv)."""
    if "antenv.axon_hooks" in sys.modules:
        return
    try:
        from trn_agent_boot.trn_boot import _ntff_profile_via_ctypes

        hook = _ntff_profile_via_ctypes("/opt/axon/libaxon_pjrt.so")
        mod = types.ModuleType("antenv.axon_hooks")
        mod.get_axon_ntff_profile_hook = lambda: hook
        sys.modules["antenv.axon_hooks"] = mod
    except Exception:
        pass


def run(inputs, T_=T, trace=False, tmpdir=None):
    """Run the sharded kernel; returns (full output [B, T, C] fp32, results obj)."""
    nc = _get_nc(T_)
    in_maps = prep_inputs(**inputs, T_=T_)
    kwargs = {}
    if trace:
        _install_trace_hook()
        kwargs = dict(trace=True, tmpdir=tmpdir)
    res = run_bass_kernel_spmd(nc, in_maps, core_ids=list(range(NCORES)), **kwargs)
    outs = [res.results[i]["out"] for i in range(NCORES)]
    full = np.stack([outs[2 * b] + outs[2 * b + 1] for b in range(B)], axis=0).astype(
        np.float32
    )
    return full, res


def kernel(x, cos, sin, w_qkv, w_proj, g_q, g_k):
    full, _ = run(
        dict(x=x, cos=cos, sin=sin, w_qkv=w_qkv, w_proj=w_proj, g_q=g_q, g_k=g_k)
    )
    return full
